# revision 11
# baseline (speedup 1.0000x reference)
"""BotRGCN Trainium2 kernel: feature transform + 2 RGCN layers + classifier.

Sharding: nodes split across 8 cores by id (12500/core, padded to 12544).
Edges partitioned by destination shard; per-core edges grouped into 4
src-bank gather streams (int16 index range), sorted by (dst-window,
relation) within each stream. Group slot quotas are uniform across cores
(max over cores) so one SPMD program serves all 8; no 128-alignment
padding — blocks may straddle group boundaries, with per-instance one-hot
masks (meta = dst-offset or -1) absorbing the mismatch.

Source features exchanged via bf16 AllGather of the per-layer node table;
per-edge rows fetched with dma_gather (4 SWDGE queues, one per bank).
Aggregation = scatter matmuls: per 128-slot block instance, a one-hot
rhs built by a single DVE tensor_scalar is_equal against a resident iota;
the per-(rel, dst) mean reciprocal is applied after aggregation via a
rank-1 broadcast matmul + elementwise multiply.
"""

import sys

sys.path.insert(0, "/opt/trn_rl_repo")

from contextlib import ExitStack

import numpy as np
import ml_dtypes

import concourse.bass as bass
import concourse.bacc as bacc
import concourse.mybir as mybir
import concourse.tile as tile
from concourse.masks import make_identity
from concourse.bass_utils import run_bass_kernel_spmd

BF16 = mybir.dt.bfloat16
F32 = mybir.dt.float32
I16 = mybir.dt.int16

P = 128

# full-problem config (test.py overrides for mini runs)
CFG = dict(
    N=100000,        # nodes
    NC=8,            # cores
    R=2,             # relations
    H=128,
    DES=768, TWEET=768, NUMP=6, CATP=11,
    WIN=128,         # dst window (PSUM free dim)
    NBLK_CH=16,      # gather-chunk size in 128-edge blocks
    BANKROWS=25088,  # gather-table bank rows (< 2^15)
    NTF=512,         # feature-stage node tile
)


def _derived(cfg):
    d = dict(cfg)
    d["SH"] = cfg["N"] // cfg["NC"]
    d["SHP"] = ((d["SH"] + P - 1) // P) * P
    d["NW"] = d["SHP"] // cfg["WIN"]
    assert d["SHP"] % cfg["WIN"] == 0
    d["TROWS"] = cfg["NC"] * d["SHP"]           # padded table rows
    d["BANKS"] = (d["TROWS"] + cfg["BANKROWS"] - 1) // cfg["BANKROWS"]
    d["TBLK"] = d["SHP"] // P                   # 128-row blobs per core
    # x feature layout: [des | tweet | num+cat packed into one 128-block]
    d["KDES"] = cfg["DES"] // P
    d["KTWEET"] = cfg["TWEET"] // P
    d["KX"] = d["KDES"] + d["KTWEET"] + 1
    d["XROWS"] = d["KX"] * P
    d["CHS"] = cfg["NBLK_CH"] * P
    return d


# ---------------------------------------------------------------------------
# host-side graph planning
# ---------------------------------------------------------------------------

class Plan:
    pass


def build_plan(edge_index, edge_type, cfg):
    """Quota-based slot layout: per (bank, window, rel) group, slot count =
    max over cores (uniform SPMD structure, no block alignment). Returns
    per-core gather-index / meta arrays plus the static instance list."""
    d = cfg
    NC, SH, SHP, WIN, NW = d["NC"], d["SH"], d["SHP"], d["WIN"], d["NW"]
    BANKS, BR, CHS = d["BANKS"], d["BANKROWS"], d["CHS"]
    R, N, TBLK = d["R"], d["N"], d["TBLK"]

    src = np.asarray(edge_index[0], dtype=np.int64)
    dst = np.asarray(edge_index[1], dtype=np.int64)
    et = np.asarray(edge_type, dtype=np.int64)

    core = dst // SH
    dl = dst - core * SH
    # table row of a (padded) node: blob layout [p][t] per shard
    sl = src - (src // SH) * SH
    ps = (src // SH) * SHP + (sl % P) * TBLK + (sl // P)
    bank = ps // BR
    bidx = (ps - bank * BR).astype(np.int16)
    w_arr = dl // WIN
    dw = (dl - w_arr * WIN).astype(np.float32)

    # per-(rel, node) in-degree -> per-core recip table [R, SHP]
    cnt = np.bincount(et * N + dst, minlength=R * N).reshape(R, N)
    recip_full = (1.0 / np.maximum(cnt, 1.0)).astype(np.float32)   # [R, N]
    recip = np.zeros((NC, R, SHP), np.float32)
    for c in range(NC):
        recip[c, :, :SH] = recip_full[:, c * SH:(c + 1) * SH]
    # [NC, NW, R*WIN]: row w holds both relations' recip for window w
    recipT = np.transpose(recip.reshape(NC, R, NW, WIN), (0, 2, 1, 3)) \
        .reshape(NC, NW, R * WIN).copy()

    # group quotas: max over cores
    NG = BANKS * NW * R
    gid = (bank * NW + w_arr) * R + et
    counts = np.bincount(core * NG + gid, minlength=NC * NG).reshape(NC, NG)
    q = counts.max(axis=0).astype(np.int64)          # [NG]

    # stream (=bank) layout: groups in (w, r) order; stream padded to chunks
    raw_len = q.reshape(BANKS, NW * R).sum(axis=1)
    pad_len = ((raw_len + CHS - 1) // CHS) * CHS
    stream_base = np.zeros(BANKS + 1, np.int64)
    np.cumsum(pad_len, out=stream_base[1:])
    TOTSLOT = int(stream_base[-1])
    gbase = np.zeros(NG, np.int64)                   # global slot base
    for b in range(BANKS):
        local = 0
        for w in range(NW):
            for r in range(R):
                g = (b * NW + w) * R + r
                gbase[g] = stream_base[b] + local
                local += int(q[g])

    # instances: (w, r, b, blk) for every block a group touches; emission
    # order (w, r, b, blk). Per group: first block + instance-id base.
    inst_list = []
    g_first_blk = np.zeros(NG, np.int64)
    g_inst_base = np.zeros(NG, np.int64)
    per_wr = [[] for _ in range(NW * R)]             # (b, blk, inst_id)
    tmp = []
    for w in range(NW):
        for r in range(R):
            for b in range(BANKS):
                g = (b * NW + w) * R + r
                if q[g] == 0:
                    g_first_blk[g] = -1
                    continue
                lb = gbase[g] - stream_base[b]
                blk0 = int(lb // P)
                blk1 = int((lb + q[g] - 1) // P)
                g_first_blk[g] = blk0
                g_inst_base[g] = len(tmp)
                for blk in range(blk0, blk1 + 1):
                    tmp.append((w, r, b, blk))
                    per_wr[w * R + r].append((b, blk, len(tmp) - 1))
    inst_list = tmp
    NINST = len(inst_list)

    # per-core placement: edges sorted stable by (core, gid), ranked in-group
    okey = core * NG + gid
    order = np.argsort(okey, kind="stable")
    so = okey[order]
    first_of = np.r_[True, so[1:] != so[:-1]]
    idx_in_run = np.arange(len(so)) - np.maximum.accumulate(
        np.where(first_of, np.arange(len(so)), 0))
    g_of = so % NG
    slot = gbase[g_of] + idx_in_run                   # global slot
    ecore = core[order]

    # gather indices: wrapped in 16 partitions, replicated for 8 core-groups
    idx16 = np.zeros((NC, P, TOTSLOT // 16), np.int16)
    col = slot // 16
    prow = (slot % 16).astype(np.int64)
    bo = bidx[order]
    for g8 in range(8):
        idx16[ecore, 16 * g8 + prow, col] = bo
    # stream-end pads: idx -1 (skipped by dma_gather). Must start at a
    # 128-block boundary: the last used block's pad slots are consumed by
    # its matmul (masked to 0 by the one-hot) so they need valid data.
    ceil_raw = ((raw_len + P - 1) // P) * P
    for b in range(BANKS):
        s0, s1 = stream_base[b] + ceil_raw[b], stream_base[b] + pad_len[b]
        if s1 > s0:
            ss = np.arange(s0, s1)
            for g8 in range(8):
                idx16[:, 16 * g8 + (ss % 16), ss // 16] = -1

    # meta: [NC, P, NINST]; dw for filled slots, -1 elsewhere
    meta = np.full((NC, P, NINST), -1.0, np.float32)
    b_of = g_of // (NW * R)
    ls = slot - stream_base[b_of]
    blk_of = ls // P
    iid = g_inst_base[g_of] + (blk_of - g_first_blk[g_of])
    meta[ecore, ls % P, iid] = dw[order]

    pl = Plan()
    pl.idx16 = idx16
    pl.meta = meta
    pl.recip = recipT.astype(ml_dtypes.bfloat16)
    pl.NINST = NINST
    pl.TOTSLOT = TOTSLOT
    pl.per_wr = per_wr
    pl.stream_base = stream_base
    pl.stream_raw = (((raw_len + P - 1) // P) * P).astype(np.int64)
    pl.stream_nchunk = (pad_len // CHS).astype(np.int64)
    return pl


def prep_x(x, cfg):
    """Per-core transposed bf16 feature blocks [XROWS, SHP]."""
    d = cfg
    NC, SH, SHP = d["NC"], d["SH"], d["SHP"]
    NUMP, TWEET, CATP, DES = d["NUMP"], d["TWEET"], d["CATP"], d["DES"]
    KD, KT = d["KDES"], d["KTWEET"]
    out = np.zeros((NC, d["XROWS"], SHP), ml_dtypes.bfloat16)
    base = (KD + KT) * P
    for c in range(NC):
        xs = x[c * SH:(c + 1) * SH]
        xT = np.zeros((d["XROWS"], SHP), np.float32)
        xT[:DES, :SH] = xs[:, NUMP + TWEET + CATP:].T
        xT[DES:DES + TWEET, :SH] = xs[:, NUMP:NUMP + TWEET].T
        xT[base:base + NUMP, :SH] = xs[:, :NUMP].T
        xT[base + 64:base + 64 + CATP, :SH] = \
            xs[:, NUMP + TWEET:NUMP + TWEET + CATP].T
        out[c] = xT.astype(ml_dtypes.bfloat16)
    return out


def prep_weights(inp, cfg):
    """bf16 weight blocks + packed fp32 biases."""
    bf = lambda a: np.asarray(a, np.float32).astype(ml_dtypes.bfloat16)
    d = cfg
    wnum = np.zeros((P, d["H"]), np.float32)
    wnum[:d["NUMP"]] = inp["W_num"]
    wcat = np.zeros((P, d["H"]), np.float32)
    wcat[64:64 + d["CATP"]] = inp["W_cat"]
    w = {
        "wdes": bf(inp["W_des"]), "wtweet": bf(inp["W_tweet"]),
        "wnum": bf(wnum), "wcat": bf(wcat), "win": bf(inp["W_in"]),
        "root1": bf(inp["root1"]), "rel10": bf(inp["rel1"][0]),
        "rel11": bf(inp["rel1"][1]),
        "root2": bf(inp["root2"]), "rel20": bf(inp["rel2"][0]),
        "rel21": bf(inp["rel2"][1]), "wcls": bf(inp["W_cls"]),
    }
    biases = np.stack(
        [inp["b_des"], inp["b_tweet"], inp["b_num"], inp["b_cat"],
         inp["b_in"], inp["prelu_a"], inp["bias1"], inp["bias2"],
         inp["b_cls"]], axis=1).astype(np.float32)   # [128, 9]
    w["biases"] = biases
    return w


# ---------------------------------------------------------------------------
# bass program
# ---------------------------------------------------------------------------

def build_bass(cfg, pl):
    d = cfg
    NC, SHP, WIN, NW, NTF = d["NC"], d["SHP"], d["WIN"], d["NW"], d["NTF"]
    BANKS, BR, CHS = d["BANKS"], d["BANKROWS"], d["CHS"]
    R, H = d["R"], d["H"]
    KD, KT, KX = d["KDES"], d["KTWEET"], d["KX"]
    TBLK = d["TBLK"]
    TROWS = d["TROWS"]
    NBLK_CH = d["NBLK_CH"]
    BPW = WIN // P          # table blocks per window (1 when WIN=128)
    assert WIN % P == 0

    nc = bacc.Bacc(None, target_bir_lowering=False, debug=False,
                   num_devices=NC, num_swdge_queues=4,
                   dynamic_dma_scratch_size=32768)

    # ---- I/O ----
    xT = nc.dram_tensor("xT", [d["XROWS"], SHP], BF16, kind="ExternalInput")
    idxt = nc.dram_tensor("idxt", [P, pl.TOTSLOT // 16], I16,
                          kind="ExternalInput")
    metat = nc.dram_tensor("metat", [P, pl.NINST], F32, kind="ExternalInput")
    recipt = nc.dram_tensor("recipt", [NW, R * WIN], BF16,
                            kind="ExternalInput")
    wts = {}
    for nm, shp in [("wdes", [d["DES"], H]), ("wtweet", [d["TWEET"], H]),
                    ("wnum", [P, H]), ("wcat", [P, H]), ("win", [4 * P, H]),
                    ("root1", [H, H]), ("rel10", [H, H]), ("rel11", [H, H]),
                    ("root2", [H, H]), ("rel20", [H, H]), ("rel21", [H, H]),
                    ("wcls", [H, H])]:
        wts[nm] = nc.dram_tensor(nm, shp, BF16, kind="ExternalInput")
    biases = nc.dram_tensor("biases", [P, 9], F32, kind="ExternalInput")
    outT = nc.dram_tensor("outT", [P, SHP], F32, kind="ExternalOutput")

    # ---- collective tables ----
    cc_in = [nc.dram_tensor(f"cc{i}_in", [SHP, H], BF16, kind="Internal")
             for i in (1, 2)]
    cc_out = [nc.dram_tensor(f"cc{i}_out", [NC * SHP, H], BF16,
                             kind="Internal", addr_space="Shared")
              for i in (1, 2)]

    rg = [list(range(NC))]

    with tile.TileContext(nc) as tc:
        with (
            tc.tile_pool(name="const", bufs=1) as cpool,
            tc.tile_pool(name="resident", bufs=1) as rpool,
            ExitStack() as mstack,
        ):
            # ---- constants ----
            ident = cpool.tile([P, P], BF16)
            make_identity(nc, ident[:])
            iota = cpool.tile([P, WIN], BF16)
            nc.gpsimd.iota(iota[:], pattern=[[1, WIN]], base=0,
                           channel_multiplier=0,
                           allow_small_or_imprecise_dtypes=True)
            ones = cpool.tile([1, P], BF16)
            nc.vector.memset(ones[:], 1.0)
            bias_t = cpool.tile([P, 9], F32)
            nc.sync.dma_start(out=bias_t[:], in_=biases[:])
            meta_sb = rpool.tile([P, pl.NINST], F32, tag="meta",
                                 name="meta", bufs=1)
            nc.sync.dma_start(out=meta_sb[:], in_=metat[:])

            wt = {}
            for nm, kb in [("wdes", KD), ("wtweet", KT), ("wnum", 1),
                           ("wcat", 1), ("win", 4), ("root1", 1),
                           ("rel10", 1), ("rel11", 1), ("root2", 1),
                           ("rel20", 1), ("rel21", 1), ("wcls", 1)]:
                t = cpool.tile([P, kb, H], BF16, tag=f"w_{nm}", name=f"w_{nm}")
                nc.sync.dma_start(
                    out=t[:], in_=wts[nm].rearrange("(k p) h -> p k h", p=P))
                wt[nm] = t

            # resident activations (transposed, [H, SHP] bf16)
            hT = [rpool.tile([P, SHP], BF16, tag="ht", name=f"hT{i}", bufs=2)
                  for i in range(2)]

            wpool = mstack.enter_context(tc.tile_pool(name="work", bufs=3))
            tpool = mstack.enter_context(
                tc.tile_pool(name="tpsum", bufs=2, space="PSUM"))

            def emit_table_block(src_hT, cc_v, blk):
                tp = tpool.tile([P, P], BF16, tag="tp", name="tp",
                                space="PSUM", bufs=2)
                nc.tensor.transpose(
                    out=tp[:], in_=src_hT[:, blk * P:(blk + 1) * P],
                    identity=ident[:])
                rowt = wpool.tile([P, P], BF16, tag="rowt", name="rowt",
                                  bufs=3)
                nc.scalar.copy(out=rowt[:], in_=tp[:])
                nc.sync.dma_start(out=cc_v[:, blk, :], in_=rowt[:])

            # =============== feature transform ===============
            fstack = ExitStack()
            fpool = fstack.enter_context(tc.tile_pool(name="featsb", bufs=2))
            fpp = fstack.enter_context(
                tc.tile_pool(name="featps", bufs=2, space="PSUM"))
            cc1_v = cc_in[0].rearrange("(p t) h -> p t h", p=P)
            ntiles = (SHP + NTF - 1) // NTF
            for t in range(ntiles):
                n0 = t * NTF
                n1 = min(SHP, n0 + NTF)
                nn = n1 - n0
                xt = fpool.tile([P, KX, NTF], BF16, tag="xt", name="xt")
                nc.sync.dma_start(
                    out=xt[:, :, :nn],
                    in_=xT.rearrange("(k p) n -> p k n", p=P)[:, :, n0:n1])

                zb = []
                pzall = fpp.tile([P, 4, NTF], F32, tag="pzall",
                                 name="pzall", space="PSUM", bufs=1)
                for bi, (wnm, ks, kn, p0, pk) in enumerate([
                        ("wdes", 0, KD, 0, P), ("wtweet", KD, KT, 0, P),
                        ("wnum", KD + KT, 1, 0, d["NUMP"]),
                        ("wcat", KD + KT, 1, 64, d["CATP"])]):
                    pz = pzall[:, bi, :]
                    for k in range(kn):
                        nc.tensor.matmul(
                            out=pz[:, :nn],
                            lhsT=wt[wnm][p0:p0 + pk, k, :],
                            rhs=xt[p0:p0 + pk, ks + k, :nn],
                            start=(k == 0), stop=(k == kn - 1))
                    v = fpool.tile([P, NTF], BF16, tag=f"v{bi}", name=f"v{bi}")
                    nc.scalar.activation(
                        out=v[:, :nn], in_=pz[:, :nn],
                        func=mybir.ActivationFunctionType.Identity,
                        bias=bias_t[:, bi:bi + 1])
                    z = fpool.tile([P, NTF], BF16, tag=f"z{bi}", name=f"z{bi}")
                    nc.vector.scalar_tensor_tensor(
                        out=z[:, :nn], in0=v[:, :nn], scalar=0.01,
                        in1=v[:, :nn], op0=mybir.AluOpType.mult,
                        op1=mybir.AluOpType.max)
                    zb.append(z)

                ph = fpp.tile([P, NTF], F32, tag="ph", name="ph", space="PSUM")
                for k in range(4):
                    nc.tensor.matmul(out=ph[:, :nn], lhsT=wt["win"][:, k, :],
                                     rhs=zb[k][:, :nn],
                                     start=(k == 0), stop=(k == 3))
                vh = fpool.tile([P, NTF], F32, tag="vh", name="vh")
                nc.scalar.activation(
                    out=vh[:, :nn], in_=ph[:, :nn],
                    func=mybir.ActivationFunctionType.Identity,
                    bias=bias_t[:, 4:5])
                nc.vector.scalar_tensor_tensor(
                    out=hT[0][:, n0:n1], in0=vh[:, :nn],
                    scalar=bias_t[:, 5:6], in1=vh[:, :nn],
                    op0=mybir.AluOpType.mult, op1=mybir.AluOpType.max)
                # emit table-1 blocks for this tile
                for blk in range(n0 // P, (n0 + nn) // P):
                    emit_table_block(hT[0], cc1_v, blk)

            fstack.close()
            ppool = mstack.enter_context(
                tc.tile_pool(name="psum", bufs=2, space="PSUM"))

            nc.gpsimd.collective_compute(
                "AllGather", mybir.AluOpType.bypass,
                ins=[cc_in[0][:]], outs=[cc_out[0][:]], replica_groups=rg)

            # =============== per-layer helper ===============
            def emit_layer(li, h_in, h_out, table, rootw, relw, bias_col,
                           fuse_cls=False, cc_v_next=None):
                cur = {}

                def ensure_chunk(b, ch):
                    if cur.get(b, (-1,))[0] == ch:
                        return cur[b][1]
                    slot0 = int(pl.stream_base[b]) + ch * CHS
                    it = wpool.tile([P, CHS // 16], I16, tag=f"idx{b}",
                                    name=f"idx{b}", bufs=3)
                    nc.sync.dma_start(
                        out=it[:],
                        in_=idxt[:, slot0 // 16:(slot0 + CHS) // 16])
                    gt = wpool.tile([P, NBLK_CH, P], BF16, tag=f"gt{b}",
                                    name=f"gt{b}", bufs=3)
                    nvalid = int(min(CHS, pl.stream_raw[b] - ch * CHS))
                    nc.gpsimd.dma_gather(
                        out_ap=gt[:],
                        in_ap=table[b * BR:min((b + 1) * BR, TROWS), :],
                        idxs_ap=it[:], num_idxs=CHS, num_idxs_reg=nvalid,
                        elem_size=H, single_packet=False, queue_num=b % 4)
                    cur[b] = (ch, gt)
                    return gt

                for w in range(NW):
                    ws = slice(w * WIN, (w + 1) * WIN)
                    # one PSUM bank: [pa_r0 | pa_r1 | rc_r0 | rc_r1]
                    parc = ppool.tile([P, 2, R, WIN], F32, tag="parc",
                                      name="parc", space="PSUM", bufs=2)
                    rcrow = wpool.tile([1, R * WIN], BF16, tag="rcrow",
                                       name="rcrow", bufs=3)
                    nc.sync.dma_start(out=rcrow[:], in_=recipt[w:w + 1, :])
                    have = []
                    for r in range(R):
                        insts = pl.per_wr[w * R + r]
                        nb = len(insts)
                        for j, (b, blk, i) in enumerate(insts):
                            gt = ensure_chunk(b, blk // NBLK_CH)
                            oh = wpool.tile([P, WIN], BF16, tag="oh",
                                            name="oh", bufs=6)
                            nc.vector.tensor_scalar(
                                out=oh[:], in0=iota[:],
                                scalar1=meta_sb[:, i:i + 1], scalar2=None,
                                op0=mybir.AluOpType.is_equal)
                            nc.tensor.matmul(
                                out=parc[:, 0, r, :],
                                lhsT=gt[:, blk % NBLK_CH, :], rhs=oh[:],
                                start=(j == 0), stop=(j == nb - 1))
                        nc.tensor.matmul(
                            out=parc[:, 1, r, :], lhsT=ones[:],
                            rhs=rcrow[:, r * WIN:(r + 1) * WIN],
                            start=True, stop=True)
                        have.append(nb > 0)

                    rcs = wpool.tile([P, R, WIN], F32, tag="rcs", name="rcs",
                                     bufs=2)
                    nc.scalar.copy(out=rcs[:], in_=parc[:, 1, :, :])
                    agg = []
                    for r in range(R):
                        asb = wpool.tile([P, WIN], BF16, tag=f"asb{r}",
                                         name=f"asb{r}", bufs=2)
                        if have[r]:
                            nc.vector.tensor_tensor(
                                out=asb[:], in0=parc[:, 0, r, :],
                                in1=rcs[:, r, :],
                                op=mybir.AluOpType.mult)
                        else:
                            nc.vector.memset(asb[:], 0.0)
                        agg.append(asb)

                    # one PSUM bank: [po | pc]
                    popc = ppool.tile([P, 2, WIN], F32, tag="popc",
                                      name="popc", space="PSUM", bufs=2)
                    po = popc[:, 0, :]
                    nc.tensor.matmul(out=po, lhsT=rootw[:, 0, :],
                                     rhs=h_in[:, ws], start=True, stop=False)
                    for r in range(R):
                        nc.tensor.matmul(out=po, lhsT=relw[r][:, 0, :],
                                         rhs=agg[r][:], start=False,
                                         stop=(r == R - 1))
                    nc.scalar.activation(
                        out=h_out[:, ws], in_=po,
                        func=mybir.ActivationFunctionType.Identity,
                        bias=bias_t[:, bias_col:bias_col + 1])
                    if cc_v_next is not None:
                        for blk in range(w * BPW, (w + 1) * BPW):
                            emit_table_block(h_out, cc_v_next, blk)
                    if fuse_cls:
                        pc = popc[:, 1, :]
                        nc.tensor.matmul(out=pc, lhsT=wt["wcls"][:, 0, :],
                                         rhs=h_out[:, ws],
                                         start=True, stop=True)
                        oc = wpool.tile([P, WIN], F32, tag="oc", name="oc",
                                        bufs=2)
                        nc.scalar.activation(
                            out=oc[:], in_=pc,
                            func=mybir.ActivationFunctionType.Identity,
                            bias=bias_t[:, 8:9])
                        nc.sync.dma_start(out=outT[:, ws], in_=oc[:])

            cc2_v = cc_in[1].rearrange("(p t) h -> p t h", p=P)
            emit_layer(0, hT[0], hT[1], cc_out[0],
                       wt["root1"], [wt["rel10"], wt["rel11"]], 6,
                       cc_v_next=cc2_v)
            nc.gpsimd.collective_compute(
                "AllGather", mybir.AluOpType.bypass,
                ins=[cc_in[1][:]], outs=[cc_out[1][:]], replica_groups=rg)
            # layer 2 writes h2 window-by-window (classifier fused);
            # third "ht" tile aliases hT[0]'s buffer (h0 is dead by then)
            h2win = rpool.tile([P, SHP], BF16, tag="ht", name="h2", bufs=2)
            emit_layer(1, hT[1], h2win, cc_out[1],
                       wt["root2"], [wt["rel20"], wt["rel21"]], 7,
                       fuse_cls=True)

    nc.compile()
    return nc


# ---------------------------------------------------------------------------
# entry point
# ---------------------------------------------------------------------------

def kernel(**inputs):
    cfg = _derived(CFG)
    return _kernel_impl(inputs, cfg)


def _kernel_impl(inputs, cfg, trace=False):
    d = cfg
    NC, SH, SHP = d["NC"], d["SH"], d["SHP"]

    pl = build_plan(inputs["edge_index"], inputs["edge_type"], d)
    xs = prep_x(np.asarray(inputs["x"], np.float32), d)
    w = prep_weights(inputs, d)

    nc = build_bass(d, pl)

    in_maps = []
    for c in range(NC):
        m = {"xT": xs[c], "idxt": pl.idx16[c], "metat": pl.meta[c],
             "recipt": pl.recip[c], "biases": w["biases"]}
        for nm in ["wdes", "wtweet", "wnum", "wcat", "win", "root1", "rel10",
                   "rel11", "root2", "rel20", "rel21", "wcls"]:
            m[nm] = w[nm]
        in_maps.append(m)

    res = run_bass_kernel_spmd(nc, in_maps, core_ids=list(range(NC)),
                               trace=trace)

    out = np.empty((NC * SH, d["H"]), np.float32)
    for c in range(NC):
        out[c * SH:(c + 1) * SH] = res.results[c]["outT"].T[:SH]
    if trace:
        return out, res
    return out


# revision 15
# speedup vs baseline: 1.4145x; 1.4145x over previous
"""BotRGCN Trainium2 kernel: feature transform + 2 RGCN layers + classifier.

Sharding: nodes split across 8 cores by id (12500/core, padded to 12544).
Edges partitioned by destination shard; per-core edges grouped into 4
src-bank gather streams (int16 index range), sorted by (dst-window,
relation) within each stream. Group slot quotas are uniform across cores
(max over cores) so one SPMD program serves all 8; no 128-alignment
padding — blocks may straddle group boundaries, with per-instance one-hot
masks (meta = dst-offset or -1) absorbing the mismatch.

Source features exchanged via bf16 AllGather of the per-layer node table;
per-edge rows fetched with dma_gather (4 SWDGE queues, one per bank).
Aggregation = scatter matmuls: per 128-slot block instance, a one-hot
rhs built by a single DVE tensor_scalar is_equal against a resident iota;
the per-(rel, dst) mean reciprocal is applied after aggregation via a
rank-1 broadcast matmul + elementwise multiply.
"""

import sys

sys.path.insert(0, "/opt/trn_rl_repo")

from contextlib import ExitStack

import numpy as np
import ml_dtypes

import concourse.bass as bass
import concourse.bacc as bacc
import concourse.mybir as mybir
import concourse.tile as tile
from concourse.masks import make_identity
from concourse.bass_utils import run_bass_kernel_spmd

BF16 = mybir.dt.bfloat16
F32 = mybir.dt.float32
I16 = mybir.dt.int16

P = 128

# full-problem config (test.py overrides for mini runs)
CFG = dict(
    N=100000,        # nodes
    NC=8,            # cores
    R=2,             # relations
    H=128,
    DES=768, TWEET=768, NUMP=6, CATP=11,
    WIN=128,         # dst window (PSUM free dim)
    NBLK_CH=16,      # gather-chunk size in 128-edge blocks
    BANKROWS=25088,  # gather-table bank rows (< 2^15)
    NTF=384,         # feature-stage node tile
)


def _derived(cfg):
    d = dict(cfg)
    d["SH"] = cfg["N"] // cfg["NC"]
    d["SHP"] = ((d["SH"] + P - 1) // P) * P
    d["NW"] = d["SHP"] // cfg["WIN"]
    assert d["SHP"] % cfg["WIN"] == 0
    d["TROWS"] = cfg["NC"] * d["SHP"]           # padded table rows
    d["BANKS"] = (d["TROWS"] + cfg["BANKROWS"] - 1) // cfg["BANKROWS"]
    d["TBLK"] = d["SHP"] // P                   # 128-row blobs per core
    # x feature layout: [des | tweet | num+cat packed into one 128-block]
    d["KDES"] = cfg["DES"] // P
    d["KTWEET"] = cfg["TWEET"] // P
    d["KX"] = d["KDES"] + d["KTWEET"] + 1
    d["XROWS"] = d["KX"] * P
    d["CHS"] = cfg["NBLK_CH"] * P
    return d


# ---------------------------------------------------------------------------
# host-side graph planning
# ---------------------------------------------------------------------------

class Plan:
    pass


def build_plan(edge_index, edge_type, cfg):
    """Quota-based slot layout: per (bank, window, rel) group, slot count =
    max over cores (uniform SPMD structure, no block alignment). Returns
    per-core gather-index / meta arrays plus the static instance list."""
    d = cfg
    NC, SH, SHP, WIN, NW = d["NC"], d["SH"], d["SHP"], d["WIN"], d["NW"]
    BANKS, BR, CHS = d["BANKS"], d["BANKROWS"], d["CHS"]
    R, N, TBLK = d["R"], d["N"], d["TBLK"]

    src = np.asarray(edge_index[0], dtype=np.int64)
    dst = np.asarray(edge_index[1], dtype=np.int64)
    et = np.asarray(edge_type, dtype=np.int64)

    core = dst // SH
    dl = dst - core * SH
    # table row of a (padded) node: blob layout [p][t] per shard
    sl = src - (src // SH) * SH
    ps = (src // SH) * SHP + (sl % P) * TBLK + (sl // P)
    bank = ps // BR
    bidx = (ps - bank * BR).astype(np.int16)
    w_arr = dl // WIN
    dw = (dl - w_arr * WIN).astype(np.float32)

    # per-(rel, node) in-degree -> per-core recip table [R, SHP]
    cnt = np.bincount(et * N + dst, minlength=R * N).reshape(R, N)
    recip_full = (1.0 / np.maximum(cnt, 1.0)).astype(np.float32)   # [R, N]
    recip = np.zeros((NC, R, SHP), np.float32)
    for c in range(NC):
        recip[c, :, :SH] = recip_full[:, c * SH:(c + 1) * SH]
    # [NC, NW, R*WIN]: row w holds both relations' recip for window w
    recipT = np.transpose(recip.reshape(NC, R, NW, WIN), (0, 2, 1, 3)) \
        .reshape(NC, NW, R * WIN).copy()

    # group quotas: max over cores
    NG = BANKS * NW * R
    gid = (bank * NW + w_arr) * R + et
    counts = np.bincount(core * NG + gid, minlength=NC * NG).reshape(NC, NG)
    q = counts.max(axis=0).astype(np.int64)          # [NG]

    # stream (=bank) layout: groups in (w, r) order; stream padded to chunks
    raw_len = q.reshape(BANKS, NW * R).sum(axis=1)
    pad_len = ((raw_len + CHS - 1) // CHS) * CHS
    stream_base = np.zeros(BANKS + 1, np.int64)
    np.cumsum(pad_len, out=stream_base[1:])
    TOTSLOT = int(stream_base[-1])
    gbase = np.zeros(NG, np.int64)                   # global slot base
    for b in range(BANKS):
        local = 0
        for w in range(NW):
            for r in range(R):
                g = (b * NW + w) * R + r
                gbase[g] = stream_base[b] + local
                local += int(q[g])

    # instances: (w, r, b, blk) for every block a group touches; emission
    # order (w, r, b, blk). Per group: first block + instance-id base.
    inst_list = []
    g_first_blk = np.zeros(NG, np.int64)
    g_inst_base = np.zeros(NG, np.int64)
    per_wr = [[] for _ in range(NW * R)]             # (b, blk, inst_id)
    tmp = []
    for w in range(NW):
        for r in range(R):
            for b in range(BANKS):
                g = (b * NW + w) * R + r
                if q[g] == 0:
                    g_first_blk[g] = -1
                    continue
                lb = gbase[g] - stream_base[b]
                blk0 = int(lb // P)
                blk1 = int((lb + q[g] - 1) // P)
                g_first_blk[g] = blk0
                g_inst_base[g] = len(tmp)
                for blk in range(blk0, blk1 + 1):
                    tmp.append((w, r, b, blk))
                    per_wr[w * R + r].append((b, blk, len(tmp) - 1))
    inst_list = tmp
    NINST = len(inst_list)

    # per-core placement: edges sorted stable by (core, gid), ranked in-group
    okey = core * NG + gid
    order = np.argsort(okey, kind="stable")
    so = okey[order]
    first_of = np.r_[True, so[1:] != so[:-1]]
    idx_in_run = np.arange(len(so)) - np.maximum.accumulate(
        np.where(first_of, np.arange(len(so)), 0))
    g_of = so % NG
    slot = gbase[g_of] + idx_in_run                   # global slot
    ecore = core[order]

    # gather indices: wrapped in 16 partitions, replicated for 8 core-groups
    idx16 = np.zeros((NC, P, TOTSLOT // 16), np.int16)
    col = slot // 16
    prow = (slot % 16).astype(np.int64)
    bo = bidx[order]
    for g8 in range(8):
        idx16[ecore, 16 * g8 + prow, col] = bo
    # stream-end pads: idx -1 (skipped by dma_gather). Must start at a
    # 128-block boundary: the last used block's pad slots are consumed by
    # its matmul (masked to 0 by the one-hot) so they need valid data.
    ceil_raw = ((raw_len + P - 1) // P) * P
    for b in range(BANKS):
        s0, s1 = stream_base[b] + ceil_raw[b], stream_base[b] + pad_len[b]
        if s1 > s0:
            ss = np.arange(s0, s1)
            for g8 in range(8):
                idx16[:, 16 * g8 + (ss % 16), ss // 16] = -1

    # meta: [NC, P, NINST]; dw for filled slots, -1 elsewhere
    meta = np.full((NC, P, NINST), -1.0, np.float32)
    b_of = g_of // (NW * R)
    ls = slot - stream_base[b_of]
    blk_of = ls // P
    iid = g_inst_base[g_of] + (blk_of - g_first_blk[g_of])
    meta[ecore, ls % P, iid] = dw[order]

    NINSTP = ((NINST + 15) // 16) * 16
    if NINSTP > NINST:
        meta = np.concatenate(
            [meta, np.full((NC, P, NINSTP - NINST), -1.0, np.float32)],
            axis=2)
    pl = Plan()
    pl.idx16 = idx16
    pl.meta = meta.astype(ml_dtypes.bfloat16)
    pl.NINSTP = NINSTP
    pl.recip = recipT.astype(ml_dtypes.bfloat16)
    pl.NINST = NINST
    pl.TOTSLOT = TOTSLOT
    pl.per_wr = per_wr
    pl.stream_base = stream_base
    pl.stream_raw = (((raw_len + P - 1) // P) * P).astype(np.int64)
    pl.stream_nchunk = (pad_len // CHS).astype(np.int64)
    return pl


def prep_x(x, cfg):
    """Per-core transposed bf16 feature blocks [XROWS, SHP]."""
    d = cfg
    NC, SH, SHP = d["NC"], d["SH"], d["SHP"]
    NUMP, TWEET, CATP, DES = d["NUMP"], d["TWEET"], d["CATP"], d["DES"]
    KD, KT = d["KDES"], d["KTWEET"]
    out = np.zeros((NC, d["XROWS"], SHP), ml_dtypes.bfloat16)
    base = (KD + KT) * P
    for c in range(NC):
        xs = x[c * SH:(c + 1) * SH]
        xT = np.zeros((d["XROWS"], SHP), np.float32)
        xT[:DES, :SH] = xs[:, NUMP + TWEET + CATP:].T
        xT[DES:DES + TWEET, :SH] = xs[:, NUMP:NUMP + TWEET].T
        xT[base:base + NUMP, :SH] = xs[:, :NUMP].T
        xT[base + 64:base + 64 + CATP, :SH] = \
            xs[:, NUMP + TWEET:NUMP + TWEET + CATP].T
        out[c] = xT.astype(ml_dtypes.bfloat16)
    return out


def prep_weights(inp, cfg):
    """bf16 weight blocks + packed fp32 biases."""
    bf = lambda a: np.asarray(a, np.float32).astype(ml_dtypes.bfloat16)
    d = cfg
    wnum = np.zeros((P, d["H"]), np.float32)
    wnum[:d["NUMP"]] = inp["W_num"]
    wcat = np.zeros((P, d["H"]), np.float32)
    wcat[64:64 + d["CATP"]] = inp["W_cat"]
    w = {
        "wdes": bf(inp["W_des"]), "wtweet": bf(inp["W_tweet"]),
        "wnum": bf(wnum), "wcat": bf(wcat), "win": bf(inp["W_in"]),
        "root1": bf(inp["root1"]), "rel10": bf(inp["rel1"][0]),
        "rel11": bf(inp["rel1"][1]),
        "root2": bf(inp["root2"]), "rel20": bf(inp["rel2"][0]),
        "rel21": bf(inp["rel2"][1]), "wcls": bf(inp["W_cls"]),
    }
    biases = np.stack(
        [inp["b_des"], inp["b_tweet"], inp["b_num"], inp["b_cat"],
         inp["b_in"], inp["prelu_a"], inp["bias1"], inp["bias2"],
         inp["b_cls"]], axis=1).astype(np.float32)   # [128, 9]
    w["biases"] = biases
    return w


# ---------------------------------------------------------------------------
# bass program
# ---------------------------------------------------------------------------

def build_bass(cfg, pl):
    d = cfg
    NC, SHP, WIN, NW, NTF = d["NC"], d["SHP"], d["WIN"], d["NW"], d["NTF"]
    BANKS, BR, CHS = d["BANKS"], d["BANKROWS"], d["CHS"]
    R, H = d["R"], d["H"]
    KD, KT, KX = d["KDES"], d["KTWEET"], d["KX"]
    TBLK = d["TBLK"]
    TROWS = d["TROWS"]
    NBLK_CH = d["NBLK_CH"]
    BPW = WIN // P          # table blocks per window (1 when WIN=128)
    assert WIN % P == 0

    nc = bacc.Bacc(None, target_bir_lowering=False, debug=False,
                   num_devices=NC, num_swdge_queues=4,
                   dynamic_dma_scratch_size=32768)

    # ---- I/O ----
    xT = nc.dram_tensor("xT", [d["XROWS"], SHP], BF16, kind="ExternalInput")
    idxt = nc.dram_tensor("idxt", [P, pl.TOTSLOT // 16], I16,
                          kind="ExternalInput")
    metat = nc.dram_tensor("metat", [P, pl.NINSTP], BF16,
                           kind="ExternalInput")
    recipt = nc.dram_tensor("recipt", [NW, R * WIN], BF16,
                            kind="ExternalInput")
    wts = {}
    for nm, shp in [("wdes", [d["DES"], H]), ("wtweet", [d["TWEET"], H]),
                    ("wnum", [P, H]), ("wcat", [P, H]), ("win", [4 * P, H]),
                    ("root1", [H, H]), ("rel10", [H, H]), ("rel11", [H, H]),
                    ("root2", [H, H]), ("rel20", [H, H]), ("rel21", [H, H]),
                    ("wcls", [H, H])]:
        wts[nm] = nc.dram_tensor(nm, shp, BF16, kind="ExternalInput")
    biases = nc.dram_tensor("biases", [P, 9], F32, kind="ExternalInput")
    outT = nc.dram_tensor("outT", [P, SHP], F32, kind="ExternalOutput")

    # ---- collective tables ----
    cc_in = [nc.dram_tensor(f"cc{i}_in", [SHP, H], BF16, kind="Internal")
             for i in (1, 2)]
    cc_out = [nc.dram_tensor(f"cc{i}_out", [NC * SHP, H], BF16,
                             kind="Internal", addr_space="Shared")
              for i in (1, 2)]

    rg = [list(range(NC))]

    with tile.TileContext(nc) as tc:
        with (
            tc.tile_pool(name="const", bufs=1) as cpool,
            tc.tile_pool(name="resident", bufs=1) as rpool,
            ExitStack() as mstack,
        ):
            # ---- constants ----
            ident = cpool.tile([P, P], BF16)
            make_identity(nc, ident[:])
            iota3 = cpool.tile([P, 16, WIN], BF16)
            nc.gpsimd.iota(iota3[:], pattern=[[0, 16], [1, WIN]], base=0,
                           channel_multiplier=0,
                           allow_small_or_imprecise_dtypes=True)
            ones = cpool.tile([1, P], BF16)
            nc.vector.memset(ones[:], 1.0)
            bias_t = cpool.tile([P, 9], F32)
            nc.sync.dma_start(out=bias_t[:], in_=biases[:])
            meta_sb = rpool.tile([P, pl.NINSTP], BF16, tag="meta",
                                 name="meta", bufs=1)
            nc.sync.dma_start(out=meta_sb[:], in_=metat[:])

            wt = {}
            for nm, kb in [("wdes", KD), ("wtweet", KT), ("wnum", 1),
                           ("wcat", 1), ("win", 4), ("root1", 1),
                           ("rel10", 1), ("rel11", 1), ("root2", 1),
                           ("rel20", 1), ("rel21", 1), ("wcls", 1)]:
                t = cpool.tile([P, kb, H], BF16, tag=f"w_{nm}", name=f"w_{nm}")
                nc.sync.dma_start(
                    out=t[:], in_=wts[nm].rearrange("(k p) h -> p k h", p=P))
                wt[nm] = t

            # resident activations (transposed, [H, SHP] bf16)
            hT = [rpool.tile([P, SHP], BF16, tag="ht", name=f"hT{i}", bufs=2)
                  for i in range(2)]

            wpool = mstack.enter_context(tc.tile_pool(name="work", bufs=3))
            tpool = mstack.enter_context(
                tc.tile_pool(name="tpsum", bufs=2, space="PSUM"))

            def emit_table_block(src_hT, cc_v, blk):
                tp = tpool.tile([P, P], BF16, tag="tp", name="tp",
                                space="PSUM", bufs=2)
                nc.tensor.transpose(
                    out=tp[:], in_=src_hT[:, blk * P:(blk + 1) * P],
                    identity=ident[:])
                rowt = wpool.tile([P, P], BF16, tag="rowt", name="rowt",
                                  bufs=3)
                nc.scalar.copy(out=rowt[:], in_=tp[:])
                nc.sync.dma_start(out=cc_v[:, blk, :], in_=rowt[:])

            # =============== feature transform ===============
            fstack = ExitStack()
            fpool = fstack.enter_context(tc.tile_pool(name="featsb", bufs=2))
            fpp = fstack.enter_context(
                tc.tile_pool(name="featps", bufs=2, space="PSUM"))
            cc1_v = cc_in[0].rearrange("(p t) h -> p t h", p=P)
            ntiles = (SHP + NTF - 1) // NTF
            for t in range(ntiles):
                n0 = t * NTF
                n1 = min(SHP, n0 + NTF)
                nn = n1 - n0
                xt = fpool.tile([P, KX, NTF], BF16, tag="xt", name="xt")
                nc.sync.dma_start(
                    out=xt[:, :, :nn],
                    in_=xT.rearrange("(k p) n -> p k n", p=P)[:, :, n0:n1])

                zb = []
                # each branch slot padded to 512 f32 = one full PSUM bank
                pzall = fpp.tile([P, 4, 512], F32, tag="pzall",
                                 name="pzall", space="PSUM", bufs=1)
                for bi, (wnm, ks, kn, p0, pk) in enumerate([
                        ("wdes", 0, KD, 0, P), ("wtweet", KD, KT, 0, P),
                        ("wnum", KD + KT, 1, 0, d["NUMP"]),
                        ("wcat", KD + KT, 1, 64, d["CATP"])]):
                    pz = pzall[:, bi, :NTF]
                    for k in range(kn):
                        nc.tensor.matmul(
                            out=pz[:, :nn],
                            lhsT=wt[wnm][p0:p0 + pk, k, :],
                            rhs=xt[p0:p0 + pk, ks + k, :nn],
                            start=(k == 0), stop=(k == kn - 1))
                    v = fpool.tile([P, NTF], BF16, tag=f"v{bi}", name=f"v{bi}")
                    nc.scalar.activation(
                        out=v[:, :nn], in_=pz[:, :nn],
                        func=mybir.ActivationFunctionType.Identity,
                        bias=bias_t[:, bi:bi + 1])
                    z = fpool.tile([P, NTF], BF16, tag=f"z{bi}", name=f"z{bi}")
                    nc.vector.scalar_tensor_tensor(
                        out=z[:, :nn], in0=v[:, :nn], scalar=0.01,
                        in1=v[:, :nn], op0=mybir.AluOpType.mult,
                        op1=mybir.AluOpType.max)
                    zb.append(z)

                ph = fpp.tile([P, NTF], F32, tag="ph", name="ph", space="PSUM")
                for k in range(4):
                    nc.tensor.matmul(out=ph[:, :nn], lhsT=wt["win"][:, k, :],
                                     rhs=zb[k][:, :nn],
                                     start=(k == 0), stop=(k == 3))
                vh = fpool.tile([P, NTF], F32, tag="vh", name="vh")
                nc.scalar.activation(
                    out=vh[:, :nn], in_=ph[:, :nn],
                    func=mybir.ActivationFunctionType.Identity,
                    bias=bias_t[:, 4:5])
                nc.vector.scalar_tensor_tensor(
                    out=hT[0][:, n0:n1], in0=vh[:, :nn],
                    scalar=bias_t[:, 5:6], in1=vh[:, :nn],
                    op0=mybir.AluOpType.mult, op1=mybir.AluOpType.max)
                # emit table-1 blocks for this tile
                for blk in range(n0 // P, (n0 + nn) // P):
                    emit_table_block(hT[0], cc1_v, blk)

            fstack.close()
            ppool = mstack.enter_context(
                tc.tile_pool(name="psum", bufs=2, space="PSUM"))

            nc.gpsimd.collective_compute(
                "AllGather", mybir.AluOpType.bypass,
                ins=[cc_in[0][:]], outs=[cc_out[0][:]], replica_groups=rg)

            # =============== per-layer helper ===============
            def emit_layer(li, h_in, h_out, table, rootw, relw, bias_col,
                           fuse_cls=False, cc_v_next=None):
                cur = {}
                cur_oh = {}

                def ensure_oh(i):
                    ch = i // 16
                    if cur_oh.get("c") == ch:
                        return cur_oh["t"]
                    i0 = ch * 16
                    oh = wpool.tile([P, 16, WIN], BF16, tag="oh",
                                    name="oh", bufs=3)
                    nc.vector.tensor_tensor(
                        out=oh[:],
                        in0=meta_sb[:, i0:i0 + 16].unsqueeze(2)
                        .to_broadcast([P, 16, WIN]),
                        in1=iota3[:],
                        op=mybir.AluOpType.is_equal)
                    cur_oh["c"] = ch
                    cur_oh["t"] = oh
                    return oh

                def ensure_chunk(b, ch):
                    if cur.get(b, (-1,))[0] == ch:
                        return cur[b][1]
                    slot0 = int(pl.stream_base[b]) + ch * CHS
                    it = wpool.tile([P, CHS // 16], I16, tag=f"idx{b}",
                                    name=f"idx{b}", bufs=3)
                    nc.sync.dma_start(
                        out=it[:],
                        in_=idxt[:, slot0 // 16:(slot0 + CHS) // 16])
                    gt = wpool.tile([P, NBLK_CH, P], BF16, tag=f"gt{b}",
                                    name=f"gt{b}", bufs=3)
                    nvalid = int(min(CHS, pl.stream_raw[b] - ch * CHS))
                    nc.gpsimd.dma_gather(
                        out_ap=gt[:],
                        in_ap=table[b * BR:min((b + 1) * BR, TROWS), :],
                        idxs_ap=it[:], num_idxs=CHS, num_idxs_reg=nvalid,
                        elem_size=H, single_packet=False, queue_num=b % 4)
                    cur[b] = (ch, gt)
                    return gt

                for w in range(NW):
                    ws = slice(w * WIN, (w + 1) * WIN)
                    # one PSUM bank: [pa_r0 | pa_r1 | rc_r0 | rc_r1]
                    parc = ppool.tile([P, 2, R, WIN], F32, tag="parc",
                                      name="parc", space="PSUM", bufs=2)
                    rcrow = wpool.tile([1, R * WIN], BF16, tag="rcrow",
                                       name="rcrow", bufs=3)
                    nc.sync.dma_start(out=rcrow[:], in_=recipt[w:w + 1, :])
                    have = []
                    for r in range(R):
                        insts = pl.per_wr[w * R + r]
                        nb = len(insts)
                        for j, (b, blk, i) in enumerate(insts):
                            gt = ensure_chunk(b, blk // NBLK_CH)
                            oh = ensure_oh(i)
                            nc.tensor.matmul(
                                out=parc[:, 0, r, :],
                                lhsT=gt[:, blk % NBLK_CH, :],
                                rhs=oh[:, i % 16, :],
                                start=(j == 0), stop=(j == nb - 1))
                        nc.tensor.matmul(
                            out=parc[:, 1, r, :], lhsT=ones[:],
                            rhs=rcrow[:, r * WIN:(r + 1) * WIN],
                            start=True, stop=True)
                        have.append(nb > 0)

                    rcs = wpool.tile([P, R, WIN], F32, tag="rcs", name="rcs",
                                     bufs=2)
                    nc.scalar.copy(out=rcs[:], in_=parc[:, 1, :, :])
                    agg = []
                    for r in range(R):
                        asb = wpool.tile([P, WIN], BF16, tag=f"asb{r}",
                                         name=f"asb{r}", bufs=2)
                        if have[r]:
                            nc.vector.tensor_tensor(
                                out=asb[:], in0=parc[:, 0, r, :],
                                in1=rcs[:, r, :],
                                op=mybir.AluOpType.mult)
                        else:
                            nc.vector.memset(asb[:], 0.0)
                        agg.append(asb)

                    # one PSUM bank: [po | pc]
                    popc = ppool.tile([P, 2, WIN], F32, tag="popc",
                                      name="popc", space="PSUM", bufs=2)
                    po = popc[:, 0, :]
                    nc.tensor.matmul(out=po, lhsT=rootw[:, 0, :],
                                     rhs=h_in[:, ws], start=True, stop=False)
                    for r in range(R):
                        nc.tensor.matmul(out=po, lhsT=relw[r][:, 0, :],
                                         rhs=agg[r][:], start=False,
                                         stop=(r == R - 1))
                    nc.scalar.activation(
                        out=h_out[:, ws], in_=po,
                        func=mybir.ActivationFunctionType.Identity,
                        bias=bias_t[:, bias_col:bias_col + 1])
                    if cc_v_next is not None:
                        for blk in range(w * BPW, (w + 1) * BPW):
                            emit_table_block(h_out, cc_v_next, blk)
                    if fuse_cls:
                        pc = popc[:, 1, :]
                        nc.tensor.matmul(out=pc, lhsT=wt["wcls"][:, 0, :],
                                         rhs=h_out[:, ws],
                                         start=True, stop=True)
                        oc = wpool.tile([P, WIN], F32, tag="oc", name="oc",
                                        bufs=2)
                        nc.scalar.activation(
                            out=oc[:], in_=pc,
                            func=mybir.ActivationFunctionType.Identity,
                            bias=bias_t[:, 8:9])
                        nc.sync.dma_start(out=outT[:, ws], in_=oc[:])

            cc2_v = cc_in[1].rearrange("(p t) h -> p t h", p=P)
            emit_layer(0, hT[0], hT[1], cc_out[0],
                       wt["root1"], [wt["rel10"], wt["rel11"]], 6,
                       cc_v_next=cc2_v)
            nc.gpsimd.collective_compute(
                "AllGather", mybir.AluOpType.bypass,
                ins=[cc_in[1][:]], outs=[cc_out[1][:]], replica_groups=rg)
            # layer 2 writes h2 window-by-window (classifier fused);
            # third "ht" tile aliases hT[0]'s buffer (h0 is dead by then)
            h2win = rpool.tile([P, SHP], BF16, tag="ht", name="h2", bufs=2)
            emit_layer(1, hT[1], h2win, cc_out[1],
                       wt["root2"], [wt["rel20"], wt["rel21"]], 7,
                       fuse_cls=True)

    nc.compile()
    return nc


# ---------------------------------------------------------------------------
# entry point
# ---------------------------------------------------------------------------

def kernel(**inputs):
    cfg = _derived(CFG)
    return _kernel_impl(inputs, cfg)


def _kernel_impl(inputs, cfg, trace=False):
    d = cfg
    NC, SH, SHP = d["NC"], d["SH"], d["SHP"]

    pl = build_plan(inputs["edge_index"], inputs["edge_type"], d)
    xs = prep_x(np.asarray(inputs["x"], np.float32), d)
    w = prep_weights(inputs, d)

    nc = build_bass(d, pl)

    in_maps = []
    for c in range(NC):
        m = {"xT": xs[c], "idxt": pl.idx16[c], "metat": pl.meta[c],
             "recipt": pl.recip[c], "biases": w["biases"]}
        for nm in ["wdes", "wtweet", "wnum", "wcat", "win", "root1", "rel10",
                   "rel11", "root2", "rel20", "rel21", "wcls"]:
            m[nm] = w[nm]
        in_maps.append(m)

    res = run_bass_kernel_spmd(nc, in_maps, core_ids=list(range(NC)),
                               trace=trace)

    out = np.empty((NC * SH, d["H"]), np.float32)
    for c in range(NC):
        out[c * SH:(c + 1) * SH] = res.results[c]["outT"].T[:SH]
    if trace:
        return out, res
    return out


# revision 17
# speedup vs baseline: 1.7304x; 1.2234x over previous
"""BotRGCN Trainium2 kernel: feature transform + 2 RGCN layers + classifier.

Sharding: nodes split across 8 cores by id (12500/core, padded to 12544).
Edges partitioned by destination shard; per-core edges grouped into 4
src-bank gather streams (int16 index range), sorted by (dst-window,
relation) within each stream. Group slot quotas are uniform across cores
(max over cores) so one SPMD program serves all 8; no 128-alignment
padding — blocks may straddle group boundaries, with per-instance one-hot
masks (meta = dst-offset or -1) absorbing the mismatch.

Source features exchanged via bf16 AllGather of the per-layer node table;
per-edge rows fetched with dma_gather (4 SWDGE queues, one per bank).
Aggregation = scatter matmuls: per 128-slot block instance, a one-hot
rhs built by a single DVE tensor_scalar is_equal against a resident iota;
the per-(rel, dst) mean reciprocal is applied after aggregation via a
rank-1 broadcast matmul + elementwise multiply.
"""

import sys

sys.path.insert(0, "/opt/trn_rl_repo")

from contextlib import ExitStack

import numpy as np
import ml_dtypes

import concourse.bass as bass
import concourse.bacc as bacc
import concourse.mybir as mybir
import concourse.tile as tile
from concourse.masks import make_identity
from concourse.bass_utils import run_bass_kernel_spmd

BF16 = mybir.dt.bfloat16
F32 = mybir.dt.float32
I16 = mybir.dt.int16

P = 128

# full-problem config (test.py overrides for mini runs)
CFG = dict(
    N=100000,        # nodes
    NC=8,            # cores
    R=2,             # relations
    H=128,
    DES=768, TWEET=768, NUMP=6, CATP=11,
    WIN=128,         # dst window (PSUM free dim)
    NBLK_CH=16,      # gather-chunk size in 128-edge blocks
    BANKROWS=25088,  # gather-table bank rows (< 2^15)
    NTF=384,         # feature-stage node tile
)


def _derived(cfg):
    d = dict(cfg)
    d["SH"] = cfg["N"] // cfg["NC"]
    d["SHP"] = ((d["SH"] + P - 1) // P) * P
    d["NW"] = d["SHP"] // cfg["WIN"]
    assert d["SHP"] % cfg["WIN"] == 0
    d["TROWS"] = cfg["NC"] * d["SHP"]           # padded table rows
    d["BANKS"] = (d["TROWS"] + cfg["BANKROWS"] - 1) // cfg["BANKROWS"]
    d["TBLK"] = d["SHP"] // P                   # 128-row blobs per core
    # x feature layout: [des | tweet | num+cat packed into one 128-block]
    d["KDES"] = cfg["DES"] // P
    d["KTWEET"] = cfg["TWEET"] // P
    d["KX"] = d["KDES"] + d["KTWEET"] + 1
    d["XROWS"] = d["KX"] * P
    d["CHS"] = cfg["NBLK_CH"] * P
    return d


# ---------------------------------------------------------------------------
# host-side graph planning
# ---------------------------------------------------------------------------

class Plan:
    pass


def build_plan(edge_index, edge_type, cfg):
    """Quota-based slot layout: per (bank, window, rel) group, slot count =
    max over cores (uniform SPMD structure, no block alignment). Returns
    per-core gather-index / meta arrays plus the static instance list."""
    d = cfg
    NC, SH, SHP, WIN, NW = d["NC"], d["SH"], d["SHP"], d["WIN"], d["NW"]
    BANKS, BR, CHS = d["BANKS"], d["BANKROWS"], d["CHS"]
    R, N, TBLK = d["R"], d["N"], d["TBLK"]

    src = np.asarray(edge_index[0], dtype=np.int64)
    dst = np.asarray(edge_index[1], dtype=np.int64)
    et = np.asarray(edge_type, dtype=np.int64)

    core = dst // SH
    dl = dst - core * SH
    # table row of a (padded) node: blob layout [p][t] per shard
    sl = src - (src // SH) * SH
    ps = (src // SH) * SHP + (sl % P) * TBLK + (sl // P)
    bank = ps // BR
    bidx = (ps - bank * BR).astype(np.int16)
    w_arr = dl // WIN
    dw = (dl - w_arr * WIN).astype(np.float32)

    # per-(rel, node) in-degree -> per-core recip table [R, SHP]
    cnt = np.bincount(et * N + dst, minlength=R * N).reshape(R, N)
    recip_full = (1.0 / np.maximum(cnt, 1.0)).astype(np.float32)   # [R, N]
    recip = np.zeros((NC, R, SHP), np.float32)
    for c in range(NC):
        recip[c, :, :SH] = recip_full[:, c * SH:(c + 1) * SH]
    # [NC, NW, R*WIN]: row w holds both relations' recip for window w
    recipT = np.transpose(recip.reshape(NC, R, NW, WIN), (0, 2, 1, 3)) \
        .reshape(NC, NW, R * WIN).copy()

    # group quotas: max over cores
    NG = BANKS * NW * R
    gid = (bank * NW + w_arr) * R + et
    counts = np.bincount(core * NG + gid, minlength=NC * NG).reshape(NC, NG)
    q = counts.max(axis=0).astype(np.int64)          # [NG]

    # stream (=bank) layout: groups in (w, r) order; stream padded to chunks
    raw_len = q.reshape(BANKS, NW * R).sum(axis=1)
    pad_len = ((raw_len + CHS - 1) // CHS) * CHS
    stream_base = np.zeros(BANKS + 1, np.int64)
    np.cumsum(pad_len, out=stream_base[1:])
    TOTSLOT = int(stream_base[-1])
    gbase = np.zeros(NG, np.int64)                   # global slot base
    for b in range(BANKS):
        local = 0
        for w in range(NW):
            for r in range(R):
                g = (b * NW + w) * R + r
                gbase[g] = stream_base[b] + local
                local += int(q[g])

    # instances: (w, r, b, blk) for every block a group touches; emission
    # order (w, r, b, blk). Per group: first block + instance-id base.
    inst_list = []
    g_first_blk = np.zeros(NG, np.int64)
    g_inst_base = np.zeros(NG, np.int64)
    per_wr = [[] for _ in range(NW * R)]             # (b, blk, inst_id)
    tmp = []
    for w in range(NW):
        for r in range(R):
            for b in range(BANKS):
                g = (b * NW + w) * R + r
                if q[g] == 0:
                    g_first_blk[g] = -1
                    continue
                lb = gbase[g] - stream_base[b]
                blk0 = int(lb // P)
                blk1 = int((lb + q[g] - 1) // P)
                g_first_blk[g] = blk0
                g_inst_base[g] = len(tmp)
                for blk in range(blk0, blk1 + 1):
                    tmp.append((w, r, b, blk))
                    per_wr[w * R + r].append((b, blk, len(tmp) - 1))
    inst_list = tmp
    NINST = len(inst_list)

    # per-core placement: edges sorted stable by (core, gid), ranked in-group
    okey = core * NG + gid
    order = np.argsort(okey, kind="stable")
    so = okey[order]
    first_of = np.r_[True, so[1:] != so[:-1]]
    idx_in_run = np.arange(len(so)) - np.maximum.accumulate(
        np.where(first_of, np.arange(len(so)), 0))
    g_of = so % NG
    slot = gbase[g_of] + idx_in_run                   # global slot
    ecore = core[order]

    # gather indices: wrapped in 16 partitions, replicated for 8 core-groups
    idx16 = np.zeros((NC, P, TOTSLOT // 16), np.int16)
    col = slot // 16
    prow = (slot % 16).astype(np.int64)
    bo = bidx[order]
    for g8 in range(8):
        idx16[ecore, 16 * g8 + prow, col] = bo
    # stream-end pads: idx -1 (skipped by dma_gather). Must start at a
    # 128-block boundary: the last used block's pad slots are consumed by
    # its matmul (masked to 0 by the one-hot) so they need valid data.
    ceil_raw = ((raw_len + P - 1) // P) * P
    for b in range(BANKS):
        s0, s1 = stream_base[b] + ceil_raw[b], stream_base[b] + pad_len[b]
        if s1 > s0:
            ss = np.arange(s0, s1)
            for g8 in range(8):
                idx16[:, 16 * g8 + (ss % 16), ss // 16] = -1

    # meta: [NC, P, NINST]; dw for filled slots, -1 elsewhere
    meta = np.full((NC, P, NINST), -1.0, np.float32)
    b_of = g_of // (NW * R)
    ls = slot - stream_base[b_of]
    blk_of = ls // P
    iid = g_inst_base[g_of] + (blk_of - g_first_blk[g_of])
    meta[ecore, ls % P, iid] = dw[order]

    NINSTP = ((NINST + 15) // 16) * 16
    if NINSTP > NINST:
        meta = np.concatenate(
            [meta, np.full((NC, P, NINSTP - NINST), -1.0, np.float32)],
            axis=2)
    pl = Plan()
    pl.idx16 = idx16
    pl.meta = meta.astype(ml_dtypes.bfloat16)
    pl.NINSTP = NINSTP
    pl.recip = recipT.astype(ml_dtypes.bfloat16)
    pl.NINST = NINST
    pl.TOTSLOT = TOTSLOT
    pl.per_wr = per_wr
    pl.stream_base = stream_base
    pl.stream_raw = (((raw_len + P - 1) // P) * P).astype(np.int64)
    pl.stream_nchunk = (pad_len // CHS).astype(np.int64)
    return pl


def prep_x(x, cfg):
    """Per-core transposed bf16 feature blocks [XROWS, SHP]."""
    d = cfg
    NC, SH, SHP = d["NC"], d["SH"], d["SHP"]
    NUMP, TWEET, CATP, DES = d["NUMP"], d["TWEET"], d["CATP"], d["DES"]
    KD, KT = d["KDES"], d["KTWEET"]
    out = np.zeros((NC, d["XROWS"], SHP), ml_dtypes.bfloat16)
    base = (KD + KT) * P
    for c in range(NC):
        xs = x[c * SH:(c + 1) * SH]
        xT = np.zeros((d["XROWS"], SHP), np.float32)
        xT[:DES, :SH] = xs[:, NUMP + TWEET + CATP:].T
        xT[DES:DES + TWEET, :SH] = xs[:, NUMP:NUMP + TWEET].T
        xT[base:base + NUMP, :SH] = xs[:, :NUMP].T
        xT[base + 64:base + 64 + CATP, :SH] = \
            xs[:, NUMP + TWEET:NUMP + TWEET + CATP].T
        out[c] = xT.astype(ml_dtypes.bfloat16)
    return out


def prep_weights(inp, cfg):
    """bf16 weight blocks + packed fp32 biases."""
    bf = lambda a: np.asarray(a, np.float32).astype(ml_dtypes.bfloat16)
    d = cfg
    wnum = np.zeros((P, d["H"]), np.float32)
    wnum[:d["NUMP"]] = inp["W_num"]
    wcat = np.zeros((P, d["H"]), np.float32)
    wcat[64:64 + d["CATP"]] = inp["W_cat"]
    w = {
        "wdes": bf(inp["W_des"]), "wtweet": bf(inp["W_tweet"]),
        "wnum": bf(wnum), "wcat": bf(wcat), "win": bf(inp["W_in"]),
        "root1": bf(inp["root1"]), "rel10": bf(inp["rel1"][0]),
        "rel11": bf(inp["rel1"][1]),
        "root2": bf(inp["root2"]), "rel20": bf(inp["rel2"][0]),
        "rel21": bf(inp["rel2"][1]), "wcls": bf(inp["W_cls"]),
    }
    biases = np.stack(
        [inp["b_des"], inp["b_tweet"], inp["b_num"], inp["b_cat"],
         inp["b_in"], inp["prelu_a"], inp["bias1"], inp["bias2"],
         inp["b_cls"]], axis=1).astype(np.float32)   # [128, 9]
    w["biases"] = biases
    return w


# ---------------------------------------------------------------------------
# bass program
# ---------------------------------------------------------------------------

def build_bass(cfg, pl):
    d = cfg
    NC, SHP, WIN, NW, NTF = d["NC"], d["SHP"], d["WIN"], d["NW"], d["NTF"]
    BANKS, BR, CHS = d["BANKS"], d["BANKROWS"], d["CHS"]
    R, H = d["R"], d["H"]
    KD, KT, KX = d["KDES"], d["KTWEET"], d["KX"]
    TBLK = d["TBLK"]
    TROWS = d["TROWS"]
    NBLK_CH = d["NBLK_CH"]
    BPW = WIN // P          # table blocks per window (1 when WIN=128)
    assert WIN % P == 0

    nc = bacc.Bacc(None, target_bir_lowering=False, debug=False,
                   num_devices=NC, num_swdge_queues=4,
                   dynamic_dma_scratch_size=32768)

    # ---- I/O ----
    xT = nc.dram_tensor("xT", [d["XROWS"], SHP], BF16, kind="ExternalInput")
    idxt = nc.dram_tensor("idxt", [P, pl.TOTSLOT // 16], I16,
                          kind="ExternalInput")
    metat = nc.dram_tensor("metat", [P, pl.NINSTP], BF16,
                           kind="ExternalInput")
    recipt = nc.dram_tensor("recipt", [NW, R * WIN], BF16,
                            kind="ExternalInput")
    wts = {}
    for nm, shp in [("wdes", [d["DES"], H]), ("wtweet", [d["TWEET"], H]),
                    ("wnum", [P, H]), ("wcat", [P, H]), ("win", [4 * P, H]),
                    ("root1", [H, H]), ("rel10", [H, H]), ("rel11", [H, H]),
                    ("root2", [H, H]), ("rel20", [H, H]), ("rel21", [H, H]),
                    ("wcls", [H, H])]:
        wts[nm] = nc.dram_tensor(nm, shp, BF16, kind="ExternalInput")
    biases = nc.dram_tensor("biases", [P, 9], F32, kind="ExternalInput")
    outT = nc.dram_tensor("outT", [P, SHP], F32, kind="ExternalOutput")

    # ---- collective tables ----
    cc_in = [nc.dram_tensor(f"cc{i}_in", [SHP, H], BF16, kind="Internal")
             for i in (1, 2)]
    cc_out = [nc.dram_tensor(f"cc{i}_out", [NC * SHP, H], BF16,
                             kind="Internal", addr_space="Shared")
              for i in (1, 2)]

    rg = [list(range(NC))]

    with tile.TileContext(nc) as tc:
        with (
            tc.tile_pool(name="const", bufs=1) as cpool,
            tc.tile_pool(name="resident", bufs=1) as rpool,
            ExitStack() as mstack,
        ):
            # ---- constants ----
            ident = cpool.tile([P, P], BF16)
            make_identity(nc, ident[:])
            iota3 = cpool.tile([P, 16, WIN], BF16)
            nc.gpsimd.iota(iota3[:], pattern=[[0, 16], [1, WIN]], base=0,
                           channel_multiplier=0,
                           allow_small_or_imprecise_dtypes=True)
            ones = cpool.tile([1, P], BF16)
            nc.vector.memset(ones[:], 1.0)
            bias_t = cpool.tile([P, 9], F32)
            nc.sync.dma_start(out=bias_t[:], in_=biases[:])
            meta_sb = rpool.tile([P, pl.NINSTP], BF16, tag="meta",
                                 name="meta", bufs=1)
            nc.sync.dma_start(out=meta_sb[:], in_=metat[:])

            wt = {}
            for nm, kb in [("wdes", KD), ("wtweet", KT), ("wnum", 1),
                           ("wcat", 1), ("win", 4), ("root1", 1),
                           ("rel10", 1), ("rel11", 1), ("root2", 1),
                           ("rel20", 1), ("rel21", 1), ("wcls", 1)]:
                t = cpool.tile([P, kb, H], BF16, tag=f"w_{nm}", name=f"w_{nm}")
                nc.sync.dma_start(
                    out=t[:], in_=wts[nm].rearrange("(k p) h -> p k h", p=P))
                wt[nm] = t

            # resident activations (transposed, [H, SHP] bf16)
            hT = [rpool.tile([P, SHP], BF16, tag="ht", name=f"hT{i}", bufs=2)
                  for i in range(2)]

            wpool = mstack.enter_context(tc.tile_pool(name="work", bufs=3))
            tpool = mstack.enter_context(
                tc.tile_pool(name="tpsum", bufs=2, space="PSUM"))

            def emit_table_block(src_hT, cc_v, blk):
                tp = tpool.tile([P, P], BF16, tag="tp", name="tp",
                                space="PSUM", bufs=2)
                nc.tensor.transpose(
                    out=tp[:], in_=src_hT[:, blk * P:(blk + 1) * P],
                    identity=ident[:])
                rowt = wpool.tile([P, P], BF16, tag="rowt", name="rowt",
                                  bufs=3)
                nc.scalar.copy(out=rowt[:], in_=tp[:])
                nc.sync.dma_start(out=cc_v[:, blk, :], in_=rowt[:])

            # =============== feature transform ===============
            fstack = ExitStack()
            fpool = fstack.enter_context(tc.tile_pool(name="featsb", bufs=2))
            fpp = fstack.enter_context(
                tc.tile_pool(name="featps", bufs=2, space="PSUM"))
            cc1_v = cc_in[0].rearrange("(p t) h -> p t h", p=P)
            ntiles = (SHP + NTF - 1) // NTF
            for t in range(ntiles):
                n0 = t * NTF
                n1 = min(SHP, n0 + NTF)
                nn = n1 - n0
                xt = fpool.tile([P, KX, NTF], BF16, tag="xt", name="xt")
                nc.sync.dma_start(
                    out=xt[:, :, :nn],
                    in_=xT.rearrange("(k p) n -> p k n", p=P)[:, :, n0:n1])

                zb = []
                # each branch slot padded to 512 f32 = one full PSUM bank
                pzall = fpp.tile([P, 4, 512], F32, tag="pzall",
                                 name="pzall", space="PSUM", bufs=1)
                for bi, (wnm, ks, kn, p0, pk) in enumerate([
                        ("wdes", 0, KD, 0, P), ("wtweet", KD, KT, 0, P),
                        ("wnum", KD + KT, 1, 0, d["NUMP"]),
                        ("wcat", KD + KT, 1, 64, d["CATP"])]):
                    pz = pzall[:, bi, :NTF]
                    for k in range(kn):
                        nc.tensor.matmul(
                            out=pz[:, :nn],
                            lhsT=wt[wnm][p0:p0 + pk, k, :],
                            rhs=xt[p0:p0 + pk, ks + k, :nn],
                            start=(k == 0), stop=(k == kn - 1))
                    v = fpool.tile([P, NTF], BF16, tag=f"v{bi}", name=f"v{bi}")
                    nc.scalar.activation(
                        out=v[:, :nn], in_=pz[:, :nn],
                        func=mybir.ActivationFunctionType.Identity,
                        bias=bias_t[:, bi:bi + 1])
                    z = fpool.tile([P, NTF], BF16, tag=f"z{bi}", name=f"z{bi}")
                    nc.vector.scalar_tensor_tensor(
                        out=z[:, :nn], in0=v[:, :nn], scalar=0.01,
                        in1=v[:, :nn], op0=mybir.AluOpType.mult,
                        op1=mybir.AluOpType.max)
                    zb.append(z)

                ph = fpp.tile([P, NTF], F32, tag="ph", name="ph", space="PSUM")
                for k in range(4):
                    nc.tensor.matmul(out=ph[:, :nn], lhsT=wt["win"][:, k, :],
                                     rhs=zb[k][:, :nn],
                                     start=(k == 0), stop=(k == 3))
                vh = fpool.tile([P, NTF], F32, tag="vh", name="vh")
                nc.scalar.activation(
                    out=vh[:, :nn], in_=ph[:, :nn],
                    func=mybir.ActivationFunctionType.Identity,
                    bias=bias_t[:, 4:5])
                nc.vector.scalar_tensor_tensor(
                    out=hT[0][:, n0:n1], in0=vh[:, :nn],
                    scalar=bias_t[:, 5:6], in1=vh[:, :nn],
                    op0=mybir.AluOpType.mult, op1=mybir.AluOpType.max)
                # emit table-1 blocks for this tile
                for blk in range(n0 // P, (n0 + nn) // P):
                    emit_table_block(hT[0], cc1_v, blk)

            fstack.close()
            ppool = mstack.enter_context(
                tc.tile_pool(name="psum", bufs=2, space="PSUM"))

            nc.gpsimd.collective_compute(
                "AllGather", mybir.AluOpType.bypass,
                ins=[cc_in[0][:]], outs=[cc_out[0][:]], replica_groups=rg)

            # =============== per-layer helper ===============
            def emit_layer(li, h_in, h_out, table, rootw, relw, bias_col,
                           fuse_cls=False, cc_v_next=None):
                cur = {}
                cur_oh = {}

                def ensure_oh(i):
                    ch = i // 16
                    if cur_oh.get("c") == ch:
                        return cur_oh["t"]
                    i0 = ch * 16
                    oh = wpool.tile([P, 16, WIN], BF16, tag="oh",
                                    name="oh", bufs=4)
                    nc.vector.tensor_tensor(
                        out=oh[:],
                        in0=meta_sb[:, i0:i0 + 16].unsqueeze(2)
                        .to_broadcast([P, 16, WIN]),
                        in1=iota3[:],
                        op=mybir.AluOpType.is_equal)
                    cur_oh["c"] = ch
                    cur_oh["t"] = oh
                    return oh

                def ensure_chunk(b, ch):
                    if cur.get(b, (-1,))[0] == ch:
                        return cur[b][1]
                    slot0 = int(pl.stream_base[b]) + ch * CHS
                    it = wpool.tile([P, CHS // 16], I16, tag=f"idx{b}",
                                    name=f"idx{b}", bufs=4)
                    nc.sync.dma_start(
                        out=it[:],
                        in_=idxt[:, slot0 // 16:(slot0 + CHS) // 16])
                    gt = wpool.tile([P, NBLK_CH, P], BF16, tag=f"gt{b}",
                                    name=f"gt{b}", bufs=3)
                    nvalid = int(min(CHS, pl.stream_raw[b] - ch * CHS))
                    nc.gpsimd.dma_gather(
                        out_ap=gt[:],
                        in_ap=table[b * BR:min((b + 1) * BR, TROWS), :],
                        idxs_ap=it[:], num_idxs=CHS, num_idxs_reg=nvalid,
                        elem_size=H, single_packet=False, queue_num=b % 4)
                    cur[b] = (ch, gt)
                    return gt

                for w in range(NW):
                    ws = slice(w * WIN, (w + 1) * WIN)
                    # one PSUM bank: [pa_r0 | pa_r1 | rc_r0 | rc_r1]
                    parc = ppool.tile([P, 2, R, WIN], F32, tag="parc",
                                      name="parc", space="PSUM", bufs=4)
                    rcrow = wpool.tile([1, R * WIN], BF16, tag="rcrow",
                                       name="rcrow", bufs=6)
                    nc.sync.dma_start(out=rcrow[:], in_=recipt[w:w + 1, :])
                    have = []
                    for r in range(R):
                        insts = pl.per_wr[w * R + r]
                        nb = len(insts)
                        for j, (b, blk, i) in enumerate(insts):
                            gt = ensure_chunk(b, blk // NBLK_CH)
                            oh = ensure_oh(i)
                            nc.tensor.matmul(
                                out=parc[:, 0, r, :],
                                lhsT=gt[:, blk % NBLK_CH, :],
                                rhs=oh[:, i % 16, :],
                                start=(j == 0), stop=(j == nb - 1))
                        nc.tensor.matmul(
                            out=parc[:, 1, r, :], lhsT=ones[:],
                            rhs=rcrow[:, r * WIN:(r + 1) * WIN],
                            start=True, stop=True)
                        have.append(nb > 0)

                    rcs = wpool.tile([P, R, WIN], F32, tag="rcs", name="rcs",
                                     bufs=4)
                    nc.scalar.copy(out=rcs[:], in_=parc[:, 1, :, :])
                    agg = []
                    for r in range(R):
                        asb = wpool.tile([P, WIN], BF16, tag=f"asb{r}",
                                         name=f"asb{r}", bufs=4)
                        if have[r]:
                            nc.vector.tensor_tensor(
                                out=asb[:], in0=parc[:, 0, r, :],
                                in1=rcs[:, r, :],
                                op=mybir.AluOpType.mult)
                        else:
                            nc.vector.memset(asb[:], 0.0)
                        agg.append(asb)

                    # one PSUM bank: [po | pc]
                    popc = ppool.tile([P, 2, WIN], F32, tag="popc",
                                      name="popc", space="PSUM", bufs=2)
                    po = popc[:, 0, :]
                    nc.tensor.matmul(out=po, lhsT=rootw[:, 0, :],
                                     rhs=h_in[:, ws], start=True, stop=False)
                    for r in range(R):
                        nc.tensor.matmul(out=po, lhsT=relw[r][:, 0, :],
                                         rhs=agg[r][:], start=False,
                                         stop=(r == R - 1))
                    nc.scalar.activation(
                        out=h_out[:, ws], in_=po,
                        func=mybir.ActivationFunctionType.Identity,
                        bias=bias_t[:, bias_col:bias_col + 1])
                    if cc_v_next is not None:
                        for blk in range(w * BPW, (w + 1) * BPW):
                            emit_table_block(h_out, cc_v_next, blk)
                    if fuse_cls:
                        pc = popc[:, 1, :]
                        nc.tensor.matmul(out=pc, lhsT=wt["wcls"][:, 0, :],
                                         rhs=h_out[:, ws],
                                         start=True, stop=True)
                        oc = wpool.tile([P, WIN], F32, tag="oc", name="oc",
                                        bufs=4)
                        nc.scalar.activation(
                            out=oc[:], in_=pc,
                            func=mybir.ActivationFunctionType.Identity,
                            bias=bias_t[:, 8:9])
                        nc.sync.dma_start(out=outT[:, ws], in_=oc[:])

            cc2_v = cc_in[1].rearrange("(p t) h -> p t h", p=P)
            emit_layer(0, hT[0], hT[1], cc_out[0],
                       wt["root1"], [wt["rel10"], wt["rel11"]], 6,
                       cc_v_next=cc2_v)
            nc.gpsimd.collective_compute(
                "AllGather", mybir.AluOpType.bypass,
                ins=[cc_in[1][:]], outs=[cc_out[1][:]], replica_groups=rg)
            # layer 2 writes h2 window-by-window (classifier fused);
            # third "ht" tile aliases hT[0]'s buffer (h0 is dead by then)
            h2win = rpool.tile([P, SHP], BF16, tag="ht", name="h2", bufs=2)
            emit_layer(1, hT[1], h2win, cc_out[1],
                       wt["root2"], [wt["rel20"], wt["rel21"]], 7,
                       fuse_cls=True)

    nc.compile()
    return nc


# ---------------------------------------------------------------------------
# entry point
# ---------------------------------------------------------------------------

def kernel(**inputs):
    cfg = _derived(CFG)
    return _kernel_impl(inputs, cfg)


def _kernel_impl(inputs, cfg, trace=False):
    d = cfg
    NC, SH, SHP = d["NC"], d["SH"], d["SHP"]

    pl = build_plan(inputs["edge_index"], inputs["edge_type"], d)
    xs = prep_x(np.asarray(inputs["x"], np.float32), d)
    w = prep_weights(inputs, d)

    nc = build_bass(d, pl)

    in_maps = []
    for c in range(NC):
        m = {"xT": xs[c], "idxt": pl.idx16[c], "metat": pl.meta[c],
             "recipt": pl.recip[c], "biases": w["biases"]}
        for nm in ["wdes", "wtweet", "wnum", "wcat", "win", "root1", "rel10",
                   "rel11", "root2", "rel20", "rel21", "wcls"]:
            m[nm] = w[nm]
        in_maps.append(m)

    res = run_bass_kernel_spmd(nc, in_maps, core_ids=list(range(NC)),
                               trace=trace)

    out = np.empty((NC * SH, d["H"]), np.float32)
    for c in range(NC):
        out[c * SH:(c + 1) * SH] = res.results[c]["outT"].T[:SH]
    if trace:
        return out, res
    return out
